# revision 8
# baseline (speedup 1.0000x reference)
"""CPSF memcell fused kernel for 8 TRN2 NeuronCores.

Memory-parallel sharding: the M=8192 memory slots are split 8 ways (1024
slots per core); every core sees the full batch B of queries and produces a
partial readout T_c = sum_{m in shard_c} gain[b,m] * T_hat[m,:].  The host
gather step sums the 8 partials (the unshard operation for an M-shard) and
transposes the [S,B] device layout back to [B,S].

Math (per core, all on device):
  w_par/w_perp = 1/max(sigma,eps)^2, w_diff = w_par - w_perp
  ||z_b - z_j||^2 = ||z_b||^2 + ||z_j||^2 - 2 z_b.z_j      (matmul form)
  proj = z_b.b_m - z_j.b_m                                  (matmul form)
  With z~ = [z, 256||z||^2, 2^-8]  (34 rows; scaling keeps fp16 normal):
    psum_J[m,b] = z~^T J = -pi*w_perp*||dz||^2
    psum_H[m,b] = z~^T H = 256*ssq*(vd.z),  ssq = sqrt(pi*max(-w_diff,0)
                                                 * ind / max(dsq, EPS^2))
    p2 = Square(2^-8*psum_H + bh),  bh = -ssq*(vd.z_j)   (ACT, one op)
    q~ = p2 + psum_J                                      (DVE, one op)
    gain16 = exp(q~ - 8ln2)            (= exp(-pi q)/256, fp16)
    T_psum += (256*alpha*T_hat)^T_tile @ gain16   (fp32 PSUM accumulate)
The MAX_Q=25 clamp is dropped: for q>25 both the clamped reference gain
(~8e-35) and ours (<=that) vanish below fp32 relevance of T.  w_diff<0 is
guaranteed by the sigma ranges (sigma_par>=0.9, sigma_perp<=0.8); the
max(-w_diff,0) guard only protects against NaN outside that regime.
"""

import os
import sys

import numpy as np

for _p in ("/opt/trn_rl_repo", "/opt/pypackages"):
    if os.path.isdir(_p) and _p not in sys.path:
        sys.path.append(_p)

B, M, N, S = 1024, 8192, 32, 128
NCORES = 8
MLOC = M // NCORES  # 1024 slots per core
P = 128             # partitions
TT = MLOC // P      # 8 m-tiles per core
BH = 512            # batch half (PSUM bank limit for fp32 free dim)
KD = N + 2          # augmented feature rows
EPS = 1e-6
TINY = float(np.finfo(np.float32).eps)
PI = float(np.pi)
R8 = 256.0          # 2^8 fp16 anti-subnormal scaling
LN2x8 = float(8.0 * np.log(2.0))

TRACE = bool(int(os.environ.get("BASS_KERNEL_TRACE", "0")))
LAST = {}           # test.py reads exec_time_ns etc. from here

_CACHE = {}


def _emit(tc):
    import concourse.bass as bass
    import concourse.mybir as mybir
    from concourse.masks import make_identity

    nc = tc.nc
    f32 = mybir.dt.float32
    f16 = mybir.dt.float16
    AF = mybir.ActivationFunctionType
    OP = mybir.AluOpType
    AX = mybir.AxisListType

    zt = nc.dram_tensor("zt", [N, B], f32, kind="ExternalInput").ap()
    zv = nc.dram_tensor("zv", [MLOC, 2 * N], f32, kind="ExternalInput").ap()
    sg = nc.dram_tensor("sg", [3, MLOC], f32, kind="ExternalInput").ap()
    th = nc.dram_tensor("th", [MLOC, S], f32, kind="ExternalInput").ap()
    tout = nc.dram_tensor("tout", [S, B], f32, kind="ExternalOutput").ap()

    with (
        tc.tile_pool(name="const", bufs=1) as const,
        tc.tile_pool(name="work", bufs=4) as work,
        tc.tile_pool(name="psw", bufs=2, space="PSUM") as psw,
        tc.tile_pool(name="pst", bufs=1, space="PSUM") as pst,
        tc.tile_pool(name="pstr", bufs=2, space="PSUM") as pstr,
    ):
        # ---- input DMAs on both HWDGE queues (SP + ACT) for parallel
        # dispatch; within each queue, ordered by first use ------------------
        zv_sb = const.tile([P, TT, 2 * N], f32, tag="zv_sb")
        nc.sync.dma_start(zv_sb[:], zv.rearrange("(p t) n -> p t n", p=P))
        th_sb = const.tile([P, TT, S], f32, tag="th_sb")
        nc.sync.dma_start(th_sb[:], th.rearrange("(p t) s -> p t s", p=P))
        sg_sb = const.tile([P, 3, TT], f32, tag="sg_sb")
        nc.scalar.dma_start(sg_sb[:], sg.rearrange("c (p t) -> p c t", p=P))
        zt_sb = const.tile([N, B], f32, tag="zt_sb")
        nc.scalar.dma_start(zt_sb[:], zt)
        zj_sb = zv_sb[:, :, 0:N]
        vd_sb = zv_sb[:, :, N:2 * N]
        sp_sb = sg_sb[:, 0, :]
        sq_sb = sg_sb[:, 1, :]
        al_sb = sg_sb[:, 2, :]

        ident = const.tile([P, P], f16, tag="ident")
        make_identity(nc, ident[:])

        # ---------------- per-slot scalars [P, TT] ----------------
        def slot(tag):
            return const.tile([P, TT], f32, tag=tag, name=tag)

        # Warm the exp table set on ACT while DMAs are in flight.
        warm = const.tile([1, 1], f32, tag="warm", name="warm")
        nc.gpsimd.memset(warm[:], 0.0)
        nc.scalar.activation(warm[:], warm[:], AF.Exp, bias=0.0, scale=1.0)

        # ---- critical chain: sigma -> w_perp / ssq -> J/H pack -----------
        wperp = slot("wperp")
        nc.vector.tensor_scalar_max(wperp[:], sq_sb[:], TINY)
        nc.vector.tensor_tensor(wperp[:], wperp[:], wperp[:], op=OP.mult)
        nc.vector.reciprocal(wperp[:], wperp[:])
        w2 = slot("w2")
        nc.vector.tensor_scalar_mul(w2[:], wperp[:], 2.0 * PI)

        wpar = slot("wpar")
        nc.vector.tensor_scalar_max(wpar[:], sp_sb[:], TINY)
        nc.vector.tensor_tensor(wpar[:], wpar[:], wpar[:], op=OP.mult)
        nc.vector.reciprocal(wpar[:], wpar[:])
        # mwd = max(w_perp - w_par, 0) = -w_diff clamped (>=0 in-dist)
        mwd = slot("mwd")
        nc.vector.tensor_tensor(mwd[:], wperp[:], wpar[:], op=OP.subtract)
        nc.vector.tensor_scalar_max(mwd[:], mwd[:], 0.0)

        tmp_n2 = const.tile([P, TT, N], f32, tag="tmp_n2")
        nc.gpsimd.tensor_tensor(tmp_n2[:], vd_sb[:], vd_sb[:], op=OP.mult)
        dsq = slot("dsq")
        nc.vector.tensor_reduce(dsq[:], tmp_n2[:], axis=AX.X, op=OP.add)
        ind = slot("ind")  # 1.0 where d_norm > EPS (== dsq > EPS^2)
        nc.vector.tensor_scalar(ind[:], dsq[:], EPS * EPS, None, op0=OP.is_gt)
        ssq = slot("ssq")  # sqrt(pi*mwd*ind/max(dsq,EPS^2))
        nc.vector.tensor_scalar_max(ssq[:], dsq[:], EPS * EPS)
        nc.vector.reciprocal(ssq[:], ssq[:])
        nc.vector.tensor_tensor(ssq[:], ssq[:], mwd[:], op=OP.mult)
        nc.vector.tensor_tensor(ssq[:], ssq[:], ind[:], op=OP.mult)
        nc.vector.tensor_scalar_mul(ssq[:], ssq[:], PI)
        nc.scalar.sqrt(ssq[:], ssq[:])

        # J pack [P, TT, KD] and H pack [P, TT, N] (slot-major, fp16)
        jp = const.tile([P, TT, KD], f16, tag="jp")
        hp = const.tile([P, TT, N], f16, tag="hp")
        nc.vector.tensor_tensor(
            jp[:, :, 0:N], zj_sb[:], w2[:, :, None].to_broadcast((P, TT, N)),
            op=OP.mult,
        )
        # j32' = -pi*w_perp/256 pairs with z~32 = 256*||z||^2
        nc.vector.tensor_scalar_mul(jp[:, :, N], wperp[:], -PI / R8)
        # -pi*w_perp*||z_j||^2 moves into the Exp's per-partition bias
        nc.gpsimd.memset(jp[:, :, N + 1], 0.0)
        # H columns: 256*ssq*vd
        hsv = const.tile([P, TT], f32, tag="hsv")
        nc.vector.tensor_scalar_mul(hsv[:], ssq[:], R8)
        nc.vector.tensor_tensor(
            hp[:], vd_sb[:],
            hsv[:, :, None].to_broadcast((P, TT, N)), op=OP.mult,
        )

        # Square bias bh = -ssq*(vd.z_j); Exp bias bexp2
        zjvd = const.tile([P, TT, N], f32, tag="zjvd")
        nc.gpsimd.tensor_tensor(zjvd[:], zj_sb[:], vd_sb[:], op=OP.mult)
        bh = slot("bh")
        nc.vector.tensor_reduce(bh[:], zjvd[:], axis=AX.X, op=OP.add)
        nc.vector.scalar_tensor_tensor(bh[:], bh[:], -1.0, ssq[:],
                                       op0=OP.mult, op1=OP.mult)

        zq = slot("zq")
        tmp_n = const.tile([P, TT, N], f32, tag="tmp_n")
        nc.gpsimd.tensor_tensor(tmp_n[:], zj_sb[:], zj_sb[:], op=OP.mult)
        nc.vector.tensor_reduce(zq[:], tmp_n[:], axis=AX.X, op=OP.add)
        nc.vector.tensor_tensor(zq[:], zq[:], wperp[:], op=OP.mult)
        bexp2 = slot("bexp2")  # -8ln2 - pi*w_perp*||zj||^2
        nc.vector.tensor_scalar(bexp2[:], zq[:], -PI, -LN2x8,
                                op0=OP.mult, op1=OP.add)

        # PE-transpose packs to feature-major (both built fp16 directly)
        jsb = const.tile([KD, TT, P], f16, tag="jsb")
        hsb = const.tile([N, TT, P], f16, tag="hsb")
        for t in range(TT):
            for k, (src, kk, dst) in enumerate(
                    ((hp, N, hsb), (jp, KD, jsb))):
                ptr = pstr.tile([P, P], f16, tag="w", name="ptr")
                nc.tensor.transpose(ptr[0:kk, 0:P], src[:, t, :], ident[:])
                if (2 * t + k) % 2 == 0:
                    nc.vector.tensor_copy(dst[:, t, :], ptr[0:kk, 0:P])
                else:
                    nc.scalar.copy(dst[:, t, :], ptr[0:kk, 0:P])

        # ---------------- z~ assembly [KD, B] fp16 --------------------------
        ztt = const.tile([KD, B], f16, tag="ztt")
        zsq = const.tile([N + 1, B], f16, tag="zsq")
        # 256*z^2 in one fused DVE op: (z*256)*z
        nc.vector.scalar_tensor_tensor(
            zsq[0:N, :], zt_sb[:], 256.0, zt_sb[:], op0=OP.mult, op1=OP.mult)
        nc.scalar.copy(ztt[0:N, :], zt_sb[:])
        nc.gpsimd.memset(zsq[N:N + 1, :], 1.0 / R8)
        # col0 sums the squares (row32 = 256||z||^2); col1 picks the constant
        # lane (row33 = 2^-8) — one matmul writes the [32:34] block.
        ones2 = const.tile([N + 1, 2], f16, tag="ones2")
        nc.gpsimd.memset(ones2[:], 0.0)
        nc.gpsimd.memset(ones2[0:N, 0:1], 1.0)
        nc.gpsimd.memset(ones2[N:N + 1, 1:2], 1.0)
        for h in range(2):
            pn = pstr.tile([P, BH], f32, tag="w")
            nc.tensor.matmul(
                pn[0:2, :], ones2[:], zsq[:, h * BH:(h + 1) * BH],
                start=True, stop=True,
            )
            nc.vector.tensor_copy(ztt[N:N + 2, h * BH:(h + 1) * BH], pn[0:2, :])

        # ---------------- T_hat * alpha * 256 -> fp16 ----------------
        th16 = const.tile([P, TT, S], f16, tag="th16")
        a2 = slot("a2")
        nc.gpsimd.tensor_scalar_mul(a2[:], al_sb[:], R8)
        for c in range(2):
            cs = slice(c * (TT // 2), (c + 1) * (TT // 2))
            nc.gpsimd.tensor_tensor(
                th16[:, cs, :], th_sb[:, cs, :],
                a2[:, cs, None].to_broadcast((P, TT // 2, S)),
                op=OP.mult,
            )

        # ---------------- main loop ----------------
        # Per m-tile: J/H matmuls (both halves) -> full-width Square (ACT)
        # -> full-width add (DVE) -> full-width Exp (ACT) -> T accumulate.
        # The T matmul for tile t-1 is emitted after tile t's J/H so the PE
        # stream is not blocked waiting on g16(t-1).
        psT = pst.tile([P, B], f32, tag="psT", name="psT")
        g16s = []
        for t in range(TT):
            pj = psw.tile([P, B], f32, tag="pjh", name="pj")
            ph = psw.tile([P, B], f32, tag="pjh", name="ph")
            for h in range(2):
                hs = slice(h * BH, (h + 1) * BH)
                nc.tensor.matmul(pj[:, hs], jsb[:, t, :], ztt[:, hs],
                                 start=True, stop=True)
                nc.tensor.matmul(ph[:, hs], hsb[:, t, :], ztt[0:N, hs],
                                 start=True, stop=True)
            if t > 0:
                for h in range(2):
                    hs = slice(h * BH, (h + 1) * BH)
                    nc.tensor.matmul(
                        psT[:, hs], th16[:, t - 1, :], g16s[t - 1][:, hs],
                        start=(t == 1), stop=False,
                    )
            g16 = work.tile([P, B], f16, tag="g")
            g16s.append(g16)
            if t < TT - 1:
                p2 = work.tile([P, B], f16, tag="p2")
                nc.scalar.activation(p2[:], ph[:], AF.Square,
                                     bias=bh[:, t:t + 1], scale=1.0 / R8)
                qt = work.tile([P, B], f32, tag="qt")
                nc.vector.tensor_tensor(qt[:], p2[:], pj[:], op=OP.add)
                nc.scalar.activation(g16[:], qt[:], AF.Exp,
                                     bias=bexp2[:, t:t + 1], scale=1.0)
            else:
                # last tile: per-half so half 0 of the output can drain
                for h in range(2):
                    hs = slice(h * BH, (h + 1) * BH)
                    p2 = work.tile([P, BH], f16, tag="p2h")
                    nc.scalar.activation(p2[:], ph[:, hs], AF.Square,
                                         bias=bh[:, t:t + 1], scale=1.0 / R8)
                    qt = work.tile([P, BH], f32, tag="qth")
                    nc.vector.tensor_tensor(qt[:], p2[:], pj[:, hs], op=OP.add)
                    nc.scalar.activation(g16[:, hs], qt[:], AF.Exp,
                                         bias=bexp2[:, t:t + 1], scale=1.0)
                    nc.tensor.matmul(psT[:, hs], th16[:, t, :], g16[:, hs],
                                     start=False, stop=(h == 1))

        # ---- drain: quarter-width copies on alternating engines, output
        # DMAs split across both HWDGE queues --------------------------------
        tsb = const.tile([P, B], f32, tag="tsb")
        BQ = B // 4
        for q in range(4):
            qs = slice(q * BQ, (q + 1) * BQ)
            if q % 2 == 0:
                nc.vector.tensor_copy(tsb[:, qs], psT[:, qs])
                nc.sync.dma_start(tout[:, qs], tsb[:, qs])
            else:
                nc.scalar.copy(tsb[:, qs], psT[:, qs])
                nc.scalar.dma_start(tout[:, qs], tsb[:, qs])


def build_nc():
    if "nc" in _CACHE:
        return _CACHE["nc"]
    import concourse.tile as tile
    from concourse import bacc

    nc = bacc.Bacc("TRN2", target_bir_lowering=False, debug=False,
                   num_devices=NCORES)
    with tile.TileContext(nc) as tc:
        _emit(tc)
    nc.compile()
    _CACHE["nc"] = nc
    return nc


def make_in_maps(z, z_j, vec_d_j, T_hat_j, alpha_j, sigma_par, sigma_perp):
    zt = np.ascontiguousarray(np.asarray(z, np.float32).T)  # layout-only
    zv = np.concatenate([np.asarray(z_j, np.float32),
                         np.asarray(vec_d_j, np.float32)], axis=1)
    sg = np.stack([np.asarray(sigma_par, np.float32),
                   np.asarray(sigma_perp, np.float32),
                   np.asarray(alpha_j, np.float32)])  # [3, M]
    in_maps = []
    for c in range(NCORES):
        s = slice(c * MLOC, (c + 1) * MLOC)
        in_maps.append({
            "zt": zt,
            "zv": np.ascontiguousarray(zv[s]),
            "sg": np.ascontiguousarray(sg[:, s]),
            "th": np.ascontiguousarray(np.asarray(T_hat_j[s], np.float32)),
        })
    return in_maps


def _run_native_cached(nc, in_maps):
    """Native (/dev/neuron*) path with a cached NEFF so repeat kernel()
    calls skip the multi-minute walrus compile that run_bass_kernel_spmd
    performs per invocation."""
    import tempfile

    from concourse import bass_utils

    if "neff" not in _CACHE:
        tmpdir = tempfile.mkdtemp(prefix="cpsf_neff_")
        _CACHE["neff"] = bass_utils.compile_bass_kernel(nc, tmpdir)
    neff_file = _CACHE["neff"]

    in_maps = [m.copy() for m in in_maps]
    out_maps = []
    for core_id, in_map in zip(range(NCORES), in_maps):
        if nc.partition_id_tensor:
            in_map[nc.partition_id_tensor.name] = np.array(
                [[core_id]], dtype=np.uint32)
        out_maps.append({"tout": np.zeros((S, B), np.float32)})
    return bass_utils.run_neff(
        neff_file, in_maps, out_maps, core_ids=list(range(NCORES)),
        has_collectives=False,
    )


def kernel(z, z_j, vec_d_j, T_hat_j, alpha_j, sigma_par, sigma_perp):
    from concourse import bass_utils
    from concourse._compat import axon_active

    nc = build_nc()
    in_maps = make_in_maps(z, z_j, vec_d_j, T_hat_j, alpha_j, sigma_par,
                           sigma_perp)
    if axon_active() or TRACE:
        res = bass_utils.run_bass_kernel_spmd(
            nc, in_maps, core_ids=list(range(NCORES)), trace=TRACE,
        )
        LAST["exec_time_ns"] = res.exec_time_ns
        LAST["mean_exec_time_ns"] = res.mean_exec_time_ns
        LAST["trace"] = res.instructions_and_trace
        results = res.results
    else:
        try:
            results = _run_native_cached(nc, in_maps)
        except Exception:
            res = bass_utils.run_bass_kernel_spmd(
                nc, in_maps, core_ids=list(range(NCORES)), trace=False,
            )
            results = res.results
    # gather: sum the 8 M-shard partials, [S,B] -> [B,S]
    acc = np.zeros((S, B), np.float64)
    for r in results:
        acc += r["tout"].astype(np.float64)
    return np.ascontiguousarray(acc.T).astype(np.float32)


# revision 13
# speedup vs baseline: 1.3417x; 1.3417x over previous
"""CPSF memcell fused kernel for 8 TRN2 NeuronCores.

Memory-parallel sharding: the M=8192 memory slots are split 8 ways (1024
slots per core); every core sees the full batch B of queries and produces a
partial readout T_c = sum_{m in shard_c} gain[b,m] * T_hat[m,:].  The host
gather step sums the 8 partials (the unshard operation for an M-shard) and
transposes the [S,B] device layout back to [B,S].

Math (per core, all on device):
  w_par/w_perp = 1/max(sigma,eps)^2, w_diff = w_par - w_perp
  ||z_b - z_j||^2 = ||z_b||^2 + ||z_j||^2 - 2 z_b.z_j      (matmul form)
  proj = z_b.b_m - z_j.b_m                                  (matmul form)
  With z~ = [z, 256||z||^2, 2^-8]  (34 rows; scaling keeps fp16 normal):
    psum_J[m,b] = z~^T J = -pi*w_perp*||dz||^2
    psum_H[m,b] = z~^T H = 256*ssq*(vd.z),  ssq = sqrt(pi*max(-w_diff,0)
                                                 * ind / max(dsq, EPS^2))
    p2 = Square(2^-8*psum_H + bh),  bh = -ssq*(vd.z_j)   (ACT, one op)
    q~ = p2 + psum_J                                      (DVE, one op)
    gain16 = exp(q~ - 8ln2)            (= exp(-pi q)/256, fp16)
    T_psum += (256*alpha*T_hat)^T_tile @ gain16   (fp32 PSUM accumulate)
The MAX_Q=25 clamp is dropped: for q>25 both the clamped reference gain
(~8e-35) and ours (<=that) vanish below fp32 relevance of T.  w_diff<0 is
guaranteed by the sigma ranges (sigma_par>=0.9, sigma_perp<=0.8); the
max(-w_diff,0) guard only protects against NaN outside that regime.
"""

import os
import sys

import numpy as np

for _p in ("/opt/trn_rl_repo", "/opt/pypackages"):
    if os.path.isdir(_p) and _p not in sys.path:
        sys.path.append(_p)

B, M, N, S = 1024, 8192, 32, 128
NCORES = 8
MLOC = M // NCORES  # 1024 slots per core
P = 128             # partitions
TT = MLOC // P      # 8 m-tiles per core
BH = 512            # batch half (PSUM bank limit for fp32 free dim)
KD = N + 2          # augmented feature rows
EPS = 1e-6
TINY = float(np.finfo(np.float32).eps)
PI = float(np.pi)
R8 = 256.0          # 2^8 fp16 anti-subnormal scaling
LN2x8 = float(8.0 * np.log(2.0))

TRACE = bool(int(os.environ.get("BASS_KERNEL_TRACE", "0")))
LAST = {}           # test.py reads exec_time_ns etc. from here

_CACHE = {}


def _emit(tc):
    import concourse.bass as bass
    import concourse.mybir as mybir
    from concourse.masks import make_identity

    nc = tc.nc
    f32 = mybir.dt.float32
    f16 = mybir.dt.float16
    AF = mybir.ActivationFunctionType
    OP = mybir.AluOpType
    AX = mybir.AxisListType

    zt = nc.dram_tensor("zt", [N, B], f32, kind="ExternalInput").ap()
    zv = nc.dram_tensor("zv", [MLOC, 2 * N], f32, kind="ExternalInput").ap()
    sg = nc.dram_tensor("sg", [3, MLOC], f32, kind="ExternalInput").ap()
    th = nc.dram_tensor("th", [MLOC, S], f32, kind="ExternalInput").ap()
    tout = nc.dram_tensor("tout", [S, B], f32, kind="ExternalOutput").ap()

    with (
        tc.tile_pool(name="const", bufs=1) as const,
        tc.tile_pool(name="work", bufs=4) as work,
        tc.tile_pool(name="psw", bufs=6, space="PSUM") as psw,
        tc.tile_pool(name="pst", bufs=1, space="PSUM") as pst,
    ):
        # ---- input DMAs on both HWDGE queues (SP + ACT) for parallel
        # dispatch; within each queue, ordered by first use ------------------
        zv_sb = const.tile([P, TT, 2 * N], f32, tag="zv_sb")
        nc.sync.dma_start(zv_sb[:], zv.rearrange("(p t) n -> p t n", p=P))
        th_sb = const.tile([P, TT, S], f32, tag="th_sb")
        nc.sync.dma_start(th_sb[:], th.rearrange("(p t) s -> p t s", p=P))
        sg_sb = const.tile([P, 3, TT], f32, tag="sg_sb")
        nc.scalar.dma_start(sg_sb[:], sg.rearrange("c (p t) -> p c t", p=P))
        zt_sb = const.tile([N, B], f32, tag="zt_sb")
        nc.scalar.dma_start(zt_sb[:], zt)
        zj_sb = zv_sb[:, :, 0:N]
        vd_sb = zv_sb[:, :, N:2 * N]
        sp_sb = sg_sb[:, 0, :]
        sq_sb = sg_sb[:, 1, :]
        al_sb = sg_sb[:, 2, :]

        ident = const.tile([P, P], f16, tag="ident")
        make_identity(nc, ident[:])

        # ---------------- per-slot scalars [P, TT] ----------------
        def slot(tag):
            return const.tile([P, TT], f32, tag=tag, name=tag)

        # Warm the exp table set on ACT while DMAs are in flight.
        warm = const.tile([1, 1], f32, tag="warm", name="warm")
        nc.gpsimd.memset(warm[:], 0.0)
        nc.scalar.activation(warm[:], warm[:], AF.Exp, bias=0.0, scale=1.0)

        # ---- H pack is sigma-independent: build it the moment zv lands ----
        hp = const.tile([P, TT, N], f16, tag="hp")
        nc.gpsimd.tensor_scalar_mul(hp[:], vd_sb[:], R8)

        # ---- sigma chain -> J pack + diag(c) Iadd weights ----------------
        wperp = slot("wperp")
        nc.vector.tensor_scalar_max(wperp[:], sq_sb[:], TINY)
        nc.vector.tensor_tensor(wperp[:], wperp[:], wperp[:], op=OP.mult)
        nc.vector.reciprocal(wperp[:], wperp[:])
        w2 = slot("w2")
        nc.vector.tensor_scalar_mul(w2[:], wperp[:], 2.0 * PI)

        jp = const.tile([P, TT, KD], f16, tag="jp")
        nc.vector.tensor_tensor(
            jp[:, :, 0:N], zj_sb[:], w2[:, :, None].to_broadcast((P, TT, N)),
            op=OP.mult,
        )
        # j32' = -pi*w_perp/256 pairs with z~32 = 256*||z||^2
        nc.vector.tensor_scalar_mul(jp[:, :, N], wperp[:], -PI / R8)
        # -pi*w_perp*||z_j||^2 moves into the Exp's per-partition bias
        nc.gpsimd.memset(jp[:, :, N + 1], 0.0)

        # c = pi*max(w_perp-w_par,0)*ind/max(dsq,EPS^2); the whole factor
        # rides the PE "Iadd" matmul as diag(c*2^-24) weights, so no sqrt
        # (and no ACT table switch away from the Exp/Square set) is needed.
        wpar = slot("wpar")
        nc.vector.tensor_scalar_max(wpar[:], sp_sb[:], TINY)
        nc.vector.tensor_tensor(wpar[:], wpar[:], wpar[:], op=OP.mult)
        nc.vector.reciprocal(wpar[:], wpar[:])
        mwd = slot("mwd")  # max(-w_diff, 0): >0 always for in-dist sigmas
        nc.vector.tensor_tensor(mwd[:], wperp[:], wpar[:], op=OP.subtract)
        nc.vector.tensor_scalar_max(mwd[:], mwd[:], 0.0)
        tmp_n2 = const.tile([P, TT, N], f32, tag="tmp_n2")
        nc.gpsimd.tensor_tensor(tmp_n2[:], vd_sb[:], vd_sb[:], op=OP.mult)
        dsq = slot("dsq")
        nc.vector.tensor_reduce(dsq[:], tmp_n2[:], axis=AX.X, op=OP.add)
        ind = slot("ind")  # 1.0 where d_norm > EPS (== dsq > EPS^2)
        nc.vector.tensor_scalar(ind[:], dsq[:], EPS * EPS, None, op0=OP.is_gt)
        cdg = slot("cdg")
        nc.vector.tensor_scalar_max(cdg[:], dsq[:], EPS * EPS)
        nc.vector.reciprocal(cdg[:], cdg[:])
        nc.vector.tensor_tensor(cdg[:], cdg[:], mwd[:], op=OP.mult)
        nc.vector.tensor_tensor(cdg[:], cdg[:], ind[:], op=OP.mult)
        c16 = const.tile([P, TT], f16, tag="c16")
        nc.vector.tensor_scalar_mul(c16[:], cdg[:], PI / float(2.0 ** 24))
        dgc = const.tile([P, TT, P], f16, tag="dgc")
        nc.vector.tensor_tensor(
            dgc[:], ident[:, None, :].to_broadcast((P, TT, P)),
            c16[:, :, None].to_broadcast((P, TT, P)), op=OP.mult,
        )

        # Square bias bh = -(vd.z_j)*2^12; Exp bias bexp2
        zjvd = const.tile([P, TT, N], f32, tag="zjvd")
        nc.gpsimd.tensor_tensor(zjvd[:], zj_sb[:], vd_sb[:], op=OP.mult)
        bh = slot("bh")
        nc.vector.tensor_reduce(bh[:], zjvd[:], axis=AX.X, op=OP.add)
        nc.vector.tensor_scalar_mul(bh[:], bh[:], -float(2.0 ** 12))

        zq = slot("zq")
        tmp_n = const.tile([P, TT, N], f32, tag="tmp_n")
        nc.gpsimd.tensor_tensor(tmp_n[:], zj_sb[:], zj_sb[:], op=OP.mult)
        nc.vector.tensor_reduce(zq[:], tmp_n[:], axis=AX.X, op=OP.add)
        nc.vector.tensor_tensor(zq[:], zq[:], wperp[:], op=OP.mult)
        bexp2 = slot("bexp2")  # -8ln2 - pi*w_perp*||zj||^2
        nc.vector.tensor_scalar(bexp2[:], zq[:], -PI, -LN2x8,
                                op0=OP.mult, op1=OP.add)

        # PE-transpose packs to feature-major (both built fp16 directly)
        jsb = const.tile([KD, TT, P], f16, tag="jsb")
        hsb = const.tile([N, TT, P], f16, tag="hsb")
        for t in range(TT):
            for k, (src, kk, dst) in enumerate(
                    ((hp, N, hsb), (jp, KD, jsb))):
                ptr = psw.tile([P, P], f16, tag="w", name="ptr")
                nc.tensor.transpose(ptr[0:kk, 0:P], src[:, t, :], ident[:])
                if (2 * t + k) % 2 == 0:
                    nc.vector.tensor_copy(dst[:, t, :], ptr[0:kk, 0:P])
                else:
                    nc.scalar.copy(dst[:, t, :], ptr[0:kk, 0:P])

        # ---------------- z~ assembly [KD, B] fp16 --------------------------
        ztt = const.tile([KD, B], f16, tag="ztt")
        zsq = const.tile([N + 1, B], f16, tag="zsq")
        # 256*z^2 in one fused DVE op: (z*256)*z
        nc.vector.scalar_tensor_tensor(
            zsq[0:N, :], zt_sb[:], 256.0, zt_sb[:], op0=OP.mult, op1=OP.mult)
        nc.scalar.copy(ztt[0:N, :], zt_sb[:])
        nc.gpsimd.memset(zsq[N:N + 1, :], 1.0 / R8)
        # col0 sums the squares (row32 = 256||z||^2); col1 picks the constant
        # lane (row33 = 2^-8) — one matmul writes the [32:34] block.
        ones2 = const.tile([N + 1, 2], f16, tag="ones2")
        nc.gpsimd.memset(ones2[:], 0.0)
        nc.gpsimd.memset(ones2[0:N, 0:1], 1.0)
        nc.gpsimd.memset(ones2[N:N + 1, 1:2], 1.0)
        for h in range(2):
            pn = psw.tile([P, BH], f32, tag="w")
            nc.tensor.matmul(
                pn[0:2, :], ones2[:], zsq[:, h * BH:(h + 1) * BH],
                start=True, stop=True,
            )
            nc.vector.tensor_copy(ztt[N:N + 2, h * BH:(h + 1) * BH], pn[0:2, :])

        # ---------------- T_hat * alpha * 256 -> fp16 ----------------
        th16 = const.tile([P, TT, S], f16, tag="th16")
        a2 = slot("a2")
        nc.gpsimd.tensor_scalar_mul(a2[:], al_sb[:], R8)
        for c in range(2):
            cs = slice(c * (TT // 2), (c + 1) * (TT // 2))
            nc.gpsimd.tensor_tensor(
                th16[:, cs, :], th_sb[:, cs, :],
                a2[:, cs, None].to_broadcast((P, TT // 2, S)),
                op=OP.mult,
            )

        # ---------------- main loop ----------------
        # 16 half-width chunks c=(t,h), software-pipelined with lag 1:
        #   iter c emits  J/H(c)  on PE,
        #                 Sq(c-1) on ACT, then Iadd(c-1) on PE
        #                 Exp(c-2) on ACT, then T(c-2) on PE.
        # Sq = (2^12*(vd.dz))^2 via ACT Square(16*ph + bh); the c factor and
        # the q~ = pj + c*p2 add both ride the PE diag(c*2^-24) matmul into
        # pj's PSUM bank, so Exp reads PSUM directly and DVE idles.
        CH = 2 * TT
        psT = pst.tile([P, B], f32, tag="psT", name="psT")
        pjs, phs, p2s, g16s = {}, {}, {}, {}
        SC12 = float(2.0 ** 12) / R8  # Square input scale: 16
        for c in range(CH + 2):
            if c < CH:
                t, h = divmod(c, 2)
                hs = slice(h * BH, (h + 1) * BH)
                pj = psw.tile([P, BH], f32, tag="w", name=f"pj{c}")
                ph = psw.tile([P, BH], f32, tag="w", name=f"ph{c}")
                pjs[c], phs[c] = pj, ph
                nc.tensor.matmul(pj[:], jsb[:, t, :], ztt[:, hs],
                                 start=True, stop=False)
                nc.tensor.matmul(ph[:], hsb[:, t, :], ztt[0:N, hs],
                                 start=True, stop=True)
            if 0 <= c - 1 < CH:
                d = c - 1
                t, h = divmod(d, 2)
                p2 = work.tile([P, BH], f16, tag="p2")
                p2s[d] = p2
                nc.scalar.activation(p2[:], phs[d][:], AF.Square,
                                     bias=bh[:, t:t + 1], scale=SC12)
                nc.tensor.matmul(pjs[d][:], dgc[:, t, :], p2[:],
                                 start=False, stop=True)
            if 0 <= c - 2 < CH:
                d = c - 2
                t, h = divmod(d, 2)
                hs = slice(h * BH, (h + 1) * BH)
                g16 = work.tile([P, BH], f16, tag="g")
                g16s[d] = g16
                nc.scalar.activation(g16[:], pjs[d][:], AF.Exp,
                                     bias=bexp2[:, t:t + 1], scale=1.0)
                nc.tensor.matmul(psT[:, hs], th16[:, t, :], g16[:],
                                 start=(t == 0), stop=(t == TT - 1))

        # ---- drain: quarter-width copies on alternating engines, output
        # DMAs split across both HWDGE queues --------------------------------
        tsb = const.tile([P, B], f32, tag="tsb")
        BQ = B // 4
        for q in range(4):
            qs = slice(q * BQ, (q + 1) * BQ)
            if q % 2 == 0:
                nc.vector.tensor_copy(tsb[:, qs], psT[:, qs])
                nc.sync.dma_start(tout[:, qs], tsb[:, qs])
            else:
                nc.scalar.copy(tsb[:, qs], psT[:, qs])
                nc.scalar.dma_start(tout[:, qs], tsb[:, qs])


def build_nc():
    if "nc" in _CACHE:
        return _CACHE["nc"]
    import concourse.tile as tile
    from concourse import bacc

    nc = bacc.Bacc("TRN2", target_bir_lowering=False, debug=False,
                   num_devices=NCORES)
    with tile.TileContext(nc) as tc:
        _emit(tc)
    nc.compile()
    _CACHE["nc"] = nc
    return nc


def make_in_maps(z, z_j, vec_d_j, T_hat_j, alpha_j, sigma_par, sigma_perp):
    zt = np.ascontiguousarray(np.asarray(z, np.float32).T)  # layout-only
    zv = np.concatenate([np.asarray(z_j, np.float32),
                         np.asarray(vec_d_j, np.float32)], axis=1)
    sg = np.stack([np.asarray(sigma_par, np.float32),
                   np.asarray(sigma_perp, np.float32),
                   np.asarray(alpha_j, np.float32)])  # [3, M]
    in_maps = []
    for c in range(NCORES):
        s = slice(c * MLOC, (c + 1) * MLOC)
        in_maps.append({
            "zt": zt,
            "zv": np.ascontiguousarray(zv[s]),
            "sg": np.ascontiguousarray(sg[:, s]),
            "th": np.ascontiguousarray(np.asarray(T_hat_j[s], np.float32)),
        })
    return in_maps


def _run_native_cached(nc, in_maps):
    """Native (/dev/neuron*) path with a cached NEFF so repeat kernel()
    calls skip the multi-minute walrus compile that run_bass_kernel_spmd
    performs per invocation."""
    import tempfile

    from concourse import bass_utils

    if "neff" not in _CACHE:
        tmpdir = tempfile.mkdtemp(prefix="cpsf_neff_")
        _CACHE["neff"] = bass_utils.compile_bass_kernel(nc, tmpdir)
    neff_file = _CACHE["neff"]

    in_maps = [m.copy() for m in in_maps]
    out_maps = []
    for core_id, in_map in zip(range(NCORES), in_maps):
        if nc.partition_id_tensor:
            in_map[nc.partition_id_tensor.name] = np.array(
                [[core_id]], dtype=np.uint32)
        out_maps.append({"tout": np.zeros((S, B), np.float32)})
    return bass_utils.run_neff(
        neff_file, in_maps, out_maps, core_ids=list(range(NCORES)),
        has_collectives=False,
    )


def kernel(z, z_j, vec_d_j, T_hat_j, alpha_j, sigma_par, sigma_perp):
    from concourse import bass_utils
    from concourse._compat import axon_active

    nc = build_nc()
    in_maps = make_in_maps(z, z_j, vec_d_j, T_hat_j, alpha_j, sigma_par,
                           sigma_perp)
    if axon_active() or TRACE:
        res = bass_utils.run_bass_kernel_spmd(
            nc, in_maps, core_ids=list(range(NCORES)), trace=TRACE,
        )
        LAST["exec_time_ns"] = res.exec_time_ns
        LAST["mean_exec_time_ns"] = res.mean_exec_time_ns
        LAST["trace"] = res.instructions_and_trace
        results = res.results
    else:
        try:
            results = _run_native_cached(nc, in_maps)
        except Exception:
            res = bass_utils.run_bass_kernel_spmd(
                nc, in_maps, core_ids=list(range(NCORES)), trace=False,
            )
            results = res.results
    # gather: sum the 8 M-shard partials, [S,B] -> [B,S]
    acc = np.zeros((S, B), np.float64)
    for r in results:
        acc += r["tout"].astype(np.float64)
    return np.ascontiguousarray(acc.T).astype(np.float32)


# revision 20
# speedup vs baseline: 1.4589x; 1.0874x over previous
"""CPSF memcell fused kernel for 8 TRN2 NeuronCores.

Memory-parallel sharding: the M=8192 memory slots are split 8 ways (1024
slots per core); every core sees the full batch B of queries and produces a
partial readout T_c = sum_{m in shard_c} gain[b,m] * T_hat[m,:].  The host
gather step sums the 8 partials (the unshard operation for an M-shard) and
transposes the [S,B] device layout back to [B,S].

Math (per core, all on device):
  w_par/w_perp = 1/max(sigma,eps)^2, w_diff = w_par - w_perp
  ||z_b - z_j||^2 = ||z_b||^2 + ||z_j||^2 - 2 z_b.z_j      (matmul form)
  proj = z_b.b_m - z_j.b_m                                  (matmul form)
  With z~ = [z, 256||z||^2, 2^-8]  (34 rows; scaling keeps fp16 normal):
    psum_J[m,b] = z~^T J = -pi*w_perp*||dz||^2
    psum_H[m,b] = z~^T H = 256*ssq*(vd.z),  ssq = sqrt(pi*max(-w_diff,0)
                                                 * ind / max(dsq, EPS^2))
    p2 = Square(2^-8*psum_H + bh),  bh = -ssq*(vd.z_j)   (ACT, one op)
    q~ = p2 + psum_J                                      (DVE, one op)
    gain16 = exp(q~ - 8ln2)            (= exp(-pi q)/256, fp16)
    T_psum += (256*alpha*T_hat)^T_tile @ gain16   (fp32 PSUM accumulate)
The MAX_Q=25 clamp is dropped: for q>25 both the clamped reference gain
(~8e-35) and ours (<=that) vanish below fp32 relevance of T.  w_diff<0 is
guaranteed by the sigma ranges (sigma_par>=0.9, sigma_perp<=0.8); the
max(-w_diff,0) guard only protects against NaN outside that regime.
"""

import os
import sys

import numpy as np

for _p in ("/opt/trn_rl_repo", "/opt/pypackages"):
    if os.path.isdir(_p) and _p not in sys.path:
        sys.path.append(_p)

B, M, N, S = 1024, 8192, 32, 128
NCORES = 8
MLOC = M // NCORES  # 1024 slots per core
P = 128             # partitions
TT = MLOC // P      # 8 m-tiles per core
BH = 512            # batch half (PSUM bank limit for fp32 free dim)
KD = N + 2          # augmented feature rows
EPS = 1e-6
TINY = float(np.finfo(np.float32).eps)
PI = float(np.pi)
R8 = 256.0          # 2^8 fp16 anti-subnormal scaling
LN2x8 = float(8.0 * np.log(2.0))

TRACE = bool(int(os.environ.get("BASS_KERNEL_TRACE", "0")))
LAST = {}           # test.py reads exec_time_ns etc. from here

_CACHE = {}


def _emit(tc):
    import concourse.bass as bass
    import concourse.mybir as mybir
    from concourse.masks import make_identity

    nc = tc.nc
    f32 = mybir.dt.float32
    f16 = mybir.dt.float16
    AF = mybir.ActivationFunctionType
    OP = mybir.AluOpType
    AX = mybir.AxisListType

    zt = nc.dram_tensor("zt", [KD, B], f16, kind="ExternalInput").ap()
    zv = nc.dram_tensor("zv", [MLOC, 2 * N], f32, kind="ExternalInput").ap()
    sg = nc.dram_tensor("sg", [3, MLOC], f32, kind="ExternalInput").ap()
    th = nc.dram_tensor("th", [MLOC, S], f32, kind="ExternalInput").ap()
    tout = nc.dram_tensor("tout", [S, B], f32, kind="ExternalOutput").ap()

    with (
        tc.tile_pool(name="const", bufs=1) as const,
        tc.tile_pool(name="work", bufs=4) as work,
        tc.tile_pool(name="psw", bufs=6, space="PSUM") as psw,
        tc.tile_pool(name="pst", bufs=1, space="PSUM") as pst,
    ):
        # ---- input DMAs on both HWDGE queues (SP + ACT) for parallel
        # dispatch; within each queue, ordered by first use.  zt arrives as
        # the fp16 z~ [KD, B] (rows N:N+2 zero-filled, completed on device).
        zv_sb = const.tile([P, TT, 2 * N], f32, tag="zv_sb")
        nc.sync.dma_start(zv_sb[:], zv.rearrange("(p t) n -> p t n", p=P))
        th_sb = const.tile([P, TT, S], f32, tag="th_sb")
        nc.sync.dma_start(th_sb[:], th.rearrange("(p t) s -> p t s", p=P))
        ztt = const.tile([KD, B], f16, tag="ztt")
        nc.scalar.dma_start(ztt[:], zt)
        sg_sb = const.tile([P, 3, TT], f32, tag="sg_sb")
        nc.scalar.dma_start(sg_sb[:], sg.rearrange("c (p t) -> p c t", p=P))
        zj_sb = zv_sb[:, :, 0:N]
        vd_sb = zv_sb[:, :, N:2 * N]
        sp_sb = sg_sb[:, 0, :]
        sq_sb = sg_sb[:, 1, :]
        al_sb = sg_sb[:, 2, :]

        ident = const.tile([P, P], f16, tag="ident")
        make_identity(nc, ident[:])

        # ---------------- per-slot scalars [P, TT] ----------------
        def slot(tag):
            return const.tile([P, TT], f32, tag=tag, name=tag)

        # Warm the exp table set on ACT while DMAs are in flight.
        warm = const.tile([1, 1], f32, tag="warm", name="warm")
        nc.gpsimd.memset(warm[:], 0.0)
        nc.scalar.activation(warm[:], warm[:], AF.Exp, bias=0.0, scale=1.0)

        # ---- H pack is sigma-independent: build it the moment zv lands ----
        hp = const.tile([P, TT, N], f16, tag="hp")
        nc.gpsimd.tensor_scalar_mul(hp[:], vd_sb[:], R8)

        # ---- sigma chain -> J pack + diag(c) Iadd weights ----------------
        wperp = slot("wperp")
        nc.vector.tensor_scalar_max(wperp[:], sq_sb[:], TINY)
        nc.vector.tensor_tensor(wperp[:], wperp[:], wperp[:], op=OP.mult)
        nc.vector.reciprocal(wperp[:], wperp[:])
        w2 = slot("w2")
        nc.vector.tensor_scalar_mul(w2[:], wperp[:], 2.0 * PI)

        jp = const.tile([P, TT, KD], f16, tag="jp")
        nc.vector.tensor_tensor(
            jp[:, :, 0:N], zj_sb[:], w2[:, :, None].to_broadcast((P, TT, N)),
            op=OP.mult,
        )
        # j32' = -pi*w_perp/256 pairs with z~32 = 256*||z||^2
        nc.vector.tensor_scalar_mul(jp[:, :, N], wperp[:], -PI / R8)
        # -pi*w_perp*||z_j||^2 moves into the Exp's per-partition bias
        nc.gpsimd.memset(jp[:, :, N + 1], 0.0)

        # c = pi*max(w_perp-w_par,0)*ind/max(dsq,EPS^2); the whole factor
        # rides the PE "Iadd" matmul as diag(c*2^-24) weights, so no sqrt
        # (and no ACT table switch away from the Exp/Square set) is needed.
        wpar = slot("wpar")
        nc.vector.tensor_scalar_max(wpar[:], sp_sb[:], TINY)
        nc.vector.tensor_tensor(wpar[:], wpar[:], wpar[:], op=OP.mult)
        nc.vector.reciprocal(wpar[:], wpar[:])
        mwd = slot("mwd")  # max(-w_diff, 0): >0 always for in-dist sigmas
        nc.vector.tensor_tensor(mwd[:], wperp[:], wpar[:], op=OP.subtract)
        nc.vector.tensor_scalar_max(mwd[:], mwd[:], 0.0)
        tmp_n2 = const.tile([P, TT, N], f32, tag="tmp_n2")
        nc.gpsimd.tensor_tensor(tmp_n2[:], vd_sb[:], vd_sb[:], op=OP.mult)
        dsq = slot("dsq")
        nc.vector.tensor_reduce(dsq[:], tmp_n2[:], axis=AX.X, op=OP.add)
        ind = slot("ind")  # 1.0 where d_norm > EPS (== dsq > EPS^2)
        nc.vector.tensor_scalar(ind[:], dsq[:], EPS * EPS, None, op0=OP.is_gt)
        cdg = slot("cdg")
        nc.vector.tensor_scalar_max(cdg[:], dsq[:], EPS * EPS)
        nc.vector.reciprocal(cdg[:], cdg[:])
        nc.vector.tensor_tensor(cdg[:], cdg[:], mwd[:], op=OP.mult)
        nc.vector.tensor_tensor(cdg[:], cdg[:], ind[:], op=OP.mult)
        c16 = slot("c16")  # f32: tensor_scalar scalar operands must be f32
        nc.vector.tensor_scalar_mul(c16[:], cdg[:], PI / float(2.0 ** 24))
        # dgc[:, t, :] = ident * c16[:, t] is built per-tile inside the main
        # loop (cheap fp16 2x tensor_scalar on an otherwise-idle DVE)
        dgc = const.tile([P, TT, P], f16, tag="dgc")

        # Square bias bh = -(vd.z_j)*2^12; Exp bias bexp2
        zjvd = const.tile([P, TT, N], f32, tag="zjvd")
        nc.gpsimd.tensor_tensor(zjvd[:], zj_sb[:], vd_sb[:], op=OP.mult)
        bh = slot("bh")
        nc.vector.tensor_reduce(bh[:], zjvd[:], axis=AX.X, op=OP.add)
        nc.vector.tensor_scalar_mul(bh[:], bh[:], -float(2.0 ** 12))

        zq = slot("zq")
        tmp_n = const.tile([P, TT, N], f32, tag="tmp_n")
        nc.gpsimd.tensor_tensor(tmp_n[:], zj_sb[:], zj_sb[:], op=OP.mult)
        nc.vector.tensor_reduce(zq[:], tmp_n[:], axis=AX.X, op=OP.add)
        nc.vector.tensor_tensor(zq[:], zq[:], wperp[:], op=OP.mult)
        bexp2 = slot("bexp2")  # -8ln2 - pi*w_perp*||zj||^2
        nc.vector.tensor_scalar(bexp2[:], zq[:], -PI, -LN2x8,
                                op0=OP.mult, op1=OP.add)

        # PE-transpose packs to feature-major (both built fp16 directly)
        jsb = const.tile([KD, TT, P], f16, tag="jsb")
        hsb = const.tile([N, TT, P], f16, tag="hsb")

        def emit_transpose(t):
            for k, (src, kk, dst) in enumerate(
                    ((hp, N, hsb), (jp, KD, jsb))):
                ptr = psw.tile([P, P], f16, tag="w", name="ptr")
                nc.tensor.transpose(ptr[0:kk, 0:P], src[:, t, :], ident[:])
                if t < 2 and k == 1:
                    nc.scalar.copy(dst[:, t, :], ptr[0:kk, 0:P])
                else:
                    nc.vector.tensor_copy(dst[:, t, :], ptr[0:kk, 0:P])

        # tiles 0-1 transposed in the preamble; 2..TT-1 stream inside the
        # main loop on PE + DVE (both have slack there)
        for t in range(2):
            emit_transpose(t)

        # ---------------- z~ tail rows (32*||z||^2, 2^-8) -------------------
        zsq = const.tile([N + 1, B], f16, tag="zsq")
        # 256*z^2 in one fused DVE op (fp16 2x): (z*256)*z
        nc.vector.scalar_tensor_tensor(
            zsq[0:N, :], ztt[0:N, :], 256.0, ztt[0:N, :],
            op0=OP.mult, op1=OP.mult)
        nc.gpsimd.memset(zsq[N:N + 1, :], 1.0 / R8)
        # col0 sums the squares (row32 = 256||z||^2); col1 picks the constant
        # lane (row33 = 2^-8) — one matmul writes the [32:34] block.
        ones2 = const.tile([N + 1, 2], f16, tag="ones2")
        nc.gpsimd.memset(ones2[:], 0.0)
        nc.gpsimd.memset(ones2[0:N, 0:1], 1.0)
        nc.gpsimd.memset(ones2[N:N + 1, 1:2], 1.0)
        for h in range(2):
            pn = psw.tile([P, BH], f32, tag="w")
            nc.tensor.matmul(
                pn[0:2, :], ones2[:], zsq[:, h * BH:(h + 1) * BH],
                start=True, stop=True,
            )
            if h == 0:
                nc.vector.tensor_copy(ztt[N:N + 2, 0:BH], pn[0:2, :])
            else:
                nc.scalar.copy(ztt[N:N + 2, BH:B], pn[0:2, :])

        # ---------------- T_hat * alpha * 256 -> fp16 ----------------
        th16 = const.tile([P, TT, S], f16, tag="th16")
        a2 = slot("a2")
        nc.gpsimd.tensor_scalar_mul(a2[:], al_sb[:], R8)
        for c in range(2):
            cs = slice(c * (TT // 2), (c + 1) * (TT // 2))
            nc.gpsimd.tensor_tensor(
                th16[:, cs, :], th_sb[:, cs, :],
                a2[:, cs, None].to_broadcast((P, TT // 2, S)),
                op=OP.mult,
            )

        # ---------------- main loop ----------------
        # 16 half-width chunks c=(t,h), software-pipelined with lag 1:
        #   iter c emits  J/H(c)  on PE,
        #                 Sq(c-1) on ACT, then Iadd(c-1) on PE
        #                 Exp(c-2) on ACT, then T(c-2) on PE.
        # Sq = (2^12*(vd.dz))^2 via ACT Square(16*ph + bh); the c factor and
        # the q~ = pj + c*p2 add both ride the PE diag(c*2^-24) matmul into
        # pj's PSUM bank, so Exp reads PSUM directly and DVE idles.
        CH = 2 * TT
        psT = pst.tile([P, B], f32, tag="psT", name="psT")
        pjs, phs, p2s, g16s = {}, {}, {}, {}
        SC12 = float(2.0 ** 12) / R8  # Square input scale: 16
        for c in range(CH + 2):
            # stream remaining pack transposes + diag(c) builds on the
            # otherwise-idle PE/DVE slack, 3 iterations ahead of first use
            tt_pre = (c + 3) // 2
            if c % 2 == 1 and 2 <= tt_pre < TT:
                emit_transpose(tt_pre)
            td = c // 2
            if c % 2 == 0 and td < TT:
                nc.vector.tensor_scalar(
                    dgc[:, td, :], ident[:], c16[:, td:td + 1], None,
                    op0=OP.mult)
            if c < CH:
                t, h = divmod(c, 2)
                hs = slice(h * BH, (h + 1) * BH)
                pj = psw.tile([P, BH], f32, tag="w", name=f"pj{c}")
                ph = psw.tile([P, BH], f32, tag="w", name=f"ph{c}")
                pjs[c], phs[c] = pj, ph
                nc.tensor.matmul(pj[:], jsb[:, t, :], ztt[:, hs],
                                 start=True, stop=False)
                nc.tensor.matmul(ph[:], hsb[:, t, :], ztt[0:N, hs],
                                 start=True, stop=True)
            if 0 <= c - 1 < CH:
                d = c - 1
                t, h = divmod(d, 2)
                p2 = work.tile([P, BH], f16, tag="p2")
                p2s[d] = p2
                nc.scalar.activation(p2[:], phs[d][:], AF.Square,
                                     bias=bh[:, t:t + 1], scale=SC12)
                nc.tensor.matmul(pjs[d][:], dgc[:, t, :], p2[:],
                                 start=False, stop=True)
            if 0 <= c - 2 < CH:
                d = c - 2
                t, h = divmod(d, 2)
                hs = slice(h * BH, (h + 1) * BH)
                g16 = work.tile([P, BH], f16, tag="g")
                g16s[d] = g16
                nc.scalar.activation(g16[:], pjs[d][:], AF.Exp,
                                     bias=bexp2[:, t:t + 1], scale=1.0)
                nc.tensor.matmul(psT[:, hs], th16[:, t, :], g16[:],
                                 start=(t == 0), stop=(t == TT - 1))
                if d == CH - 2:
                    # half 0 of psT is closed: drain it while the last
                    # chunk computes (separate tiles avoid false deps)
                    for q in range(2):
                        qs = slice(q * (BH // 2), (q + 1) * (BH // 2))
                        tq = const.tile([P, BH // 2], f32, tag=f"tq{q}")
                        nc.vector.tensor_copy(tq[:], psT[:, qs])
                        if q == 0:
                            nc.sync.dma_start(tout[:, qs], tq[:])
                        else:
                            nc.scalar.dma_start(tout[:, qs], tq[:])

        # ---- drain half 1: quarter copies on both engines, both queues ----
        for q in range(2, 4):
            qs = slice(q * (BH // 2), (q + 1) * (BH // 2))
            tq = const.tile([P, BH // 2], f32, tag=f"tq{q}")
            if q == 2:
                nc.vector.tensor_copy(tq[:], psT[:, qs])
                nc.sync.dma_start(tout[:, qs], tq[:])
            else:
                nc.scalar.copy(tq[:], psT[:, qs])
                nc.scalar.dma_start(tout[:, qs], tq[:])


def build_nc():
    if "nc" in _CACHE:
        return _CACHE["nc"]
    import concourse.tile as tile
    from concourse import bacc

    nc = bacc.Bacc("TRN2", target_bir_lowering=False, debug=False,
                   num_devices=NCORES)
    with tile.TileContext(nc) as tc:
        _emit(tc)
    nc.compile()
    _CACHE["nc"] = nc
    return nc


def make_in_maps(z, z_j, vec_d_j, T_hat_j, alpha_j, sigma_par, sigma_perp):
    # z~ rows 0:N as fp16 (layout/dtype only); rows N:N+2 are completed on
    # device (256||z||^2 via the ones2 matmul, and the 2^-8 constant lane).
    zt = np.zeros((KD, B), np.float16)
    zt[0:N] = np.asarray(z, np.float32).T.astype(np.float16)
    zv = np.concatenate([np.asarray(z_j, np.float32),
                         np.asarray(vec_d_j, np.float32)], axis=1)
    sg = np.stack([np.asarray(sigma_par, np.float32),
                   np.asarray(sigma_perp, np.float32),
                   np.asarray(alpha_j, np.float32)])  # [3, M]
    in_maps = []
    for c in range(NCORES):
        s = slice(c * MLOC, (c + 1) * MLOC)
        in_maps.append({
            "zt": zt,
            "zv": np.ascontiguousarray(zv[s]),
            "sg": np.ascontiguousarray(sg[:, s]),
            "th": np.ascontiguousarray(np.asarray(T_hat_j[s], np.float32)),
        })
    return in_maps


def _run_native_cached(nc, in_maps):
    """Native (/dev/neuron*) path with a cached NEFF so repeat kernel()
    calls skip the multi-minute walrus compile that run_bass_kernel_spmd
    performs per invocation."""
    import tempfile

    from concourse import bass_utils

    if "neff" not in _CACHE:
        tmpdir = tempfile.mkdtemp(prefix="cpsf_neff_")
        _CACHE["neff"] = bass_utils.compile_bass_kernel(nc, tmpdir)
    neff_file = _CACHE["neff"]

    in_maps = [m.copy() for m in in_maps]
    out_maps = []
    for core_id, in_map in zip(range(NCORES), in_maps):
        if nc.partition_id_tensor:
            in_map[nc.partition_id_tensor.name] = np.array(
                [[core_id]], dtype=np.uint32)
        out_maps.append({"tout": np.zeros((S, B), np.float32)})
    return bass_utils.run_neff(
        neff_file, in_maps, out_maps, core_ids=list(range(NCORES)),
        has_collectives=False,
    )


def kernel(z, z_j, vec_d_j, T_hat_j, alpha_j, sigma_par, sigma_perp):
    from concourse import bass_utils
    from concourse._compat import axon_active

    nc = build_nc()
    in_maps = make_in_maps(z, z_j, vec_d_j, T_hat_j, alpha_j, sigma_par,
                           sigma_perp)
    if axon_active() or TRACE:
        res = bass_utils.run_bass_kernel_spmd(
            nc, in_maps, core_ids=list(range(NCORES)), trace=TRACE,
        )
        LAST["exec_time_ns"] = res.exec_time_ns
        LAST["mean_exec_time_ns"] = res.mean_exec_time_ns
        LAST["trace"] = res.instructions_and_trace
        results = res.results
    else:
        try:
            results = _run_native_cached(nc, in_maps)
        except Exception:
            res = bass_utils.run_bass_kernel_spmd(
                nc, in_maps, core_ids=list(range(NCORES)), trace=False,
            )
            results = res.results
    # gather: sum the 8 M-shard partials, [S,B] -> [B,S]
    acc = np.zeros((S, B), np.float64)
    for r in results:
        acc += r["tout"].astype(np.float64)
    return np.ascontiguousarray(acc.T).astype(np.float32)
